# revision 49
# baseline (speedup 1.0000x reference)
"""GAU (gated attention unit) Trainium2 Bass kernel, 8-core SPMD.

Problem: B=4, T=2048, D=1024, DF=2048, S=128, fp32 in/out.
  u = silu(x@Wu+bu); v = silu(x@Wv+bv); z = silu(x@Wqk+bqk)
  q = (z*g0+b0)/sqrt(S); k = z*g1+b1
  scores = (q+u_qk) @ k^T, key-masked by length[b]; attn = softmax
  out = u * (attn@v); y = out@Wo + bo

Sharding: core c -> batch b=c//2, token "half" h=c%2, where the halves
INTERLEAVE at 64-token granularity: core h owns the 64-token blocks
{2a+h : a=0..15} of its batch.  With keys laid out in (nearly) global
token order, the valid keys (key pos < length[b], length >= T/2) form a
PREFIX of the 128-key tiles on every core, so the whole attention side
(scores, exp, attn@v, denominators) statically runs over only
NJ = max_b ceil(length[b]/128) tiles instead of T/128 = 16 -- the
masked tail provably contributes exp(-inf)=0.  NJ is read from
`length` at kernel() time and compiled in (cached per NJ), and the v/k
projections trim to the NJ valid 128-token chunks as well (all own
tokens are still queries, so u/z/q/y stay full).

Key tile a (128 keys) = global tokens [128a, 128a+128): own block a at
partitions [0:64) if a even else [64:128), partner block a at the
complement.  This placement is SPMD-static: every core computes v/k
for its own blocks into the same slots and receives the partner's via
the same pair-wise "sum exchange" (ReduceScatter(add) with
host-supplied one-hot shard masks) -- only the host-built mask and the
input token permutation differ per core.  Attention is key-permutation
invariant; the host un-permutes the output rows.

Layout strategy (everything stays transposed so no on-device
transposes are needed; host pre-transposes x and pre-packs weights):
  xtT  [d, tq]  : rhs/lhsT for all projections (contract d on partitions)
  z    [s, tq]  = (Wqk lhsT) @ (xtT rhs)         -> qT [s,tq], k own
  v    [tk, f]  = (xtT lhsT) @ (Wv rhs)          own 64-blocks -> slots
  uT   [f, tq]  = (Wu lhsT) @ (xtT rhs)
  pT   [tk,tq]  = exp((kT lhsT)@(qT rhs) + mask) softmax numerator
  den  [tq, 1]  = (pT lhsT) @ (ones rhs)         per-query denominator
  oT   [f, tq]  = (v lhsT) @ (pT rhs), gated *uT
  y    [tq, d]  = (oT lhsT) @ (Wo rhs), *1/den, +bo

All matmuls in bf16 with fp32 PSUM accumulation. Softmax skips the
row-max subtraction: pre-softmax logits for this operator are
|s| <~ 2, so exp() cannot overflow; masked keys get a -1e30 bias ->
exp==0 exactly.

Schedule highlights (cost-model-driven; core 0 CoreSim = the metric):
 - scores+exp+attnv(ftg0) are FUSED per query chunk: the attnv j-groups
   trail the scores by 3, so the Exp act-table-load latency and the
   exp cadence hide under attnv matmuls instead of filler matmuls
   (the PE p-state ramp is a no-op in this cost model, so the old
   warm-fill matmuls were pure loss).
 - THREE collectives (15us fixed launch each, serialized on the
   collective cores): [v f0:1024 + k] then [f1024:1536] then
   [f1536:2048], sized so each lands just before its attnv consumer.
 - v/k readbacks split so key tiles j0..j3 arrive first; the attnv
   j-loop consumes tiles in order.
 - DMAs spread across the sync/scalar/gpsimd queues (a DMA's cost is
   charged to the issuing engine's queue; collectives occupy their
   issuing queue for their whole duration, so they own gpsimd).
"""

import numpy as np
import ml_dtypes

B, T, D, DF, S = 4, 2048, 1024, 2048, 128
TQ = T // 2  # tokens per core (query count == owned kv token count)
N_CORES = 8
BF16 = ml_dtypes.bfloat16
PAIRS = [[0, 1], [2, 3], [4, 5], [6, 7]]

_NC = {}


def _build_nc(NJ, with_vbias=True, with_obias=True):
    import concourse.mybir as mybir
    import concourse.tile as tile
    from concourse import bacc
    from concourse.bass import ts, ds

    f32 = mybir.dt.float32
    bf16 = mybir.dt.bfloat16
    f8 = mybir.dt.float8e4
    AF = mybir.ActivationFunctionType
    OP = mybir.AluOpType

    NJV = (NJ + 1) // 2  # own-token 128-chunks == even-block count
    NO = NJ // 2         # odd-block count

    nc = bacc.Bacc("TRN2", dynamic_dma_scratch_size=4096)

    # ---- I/O ----
    xt_d = nc.dram_tensor("xt", [128, 8, TQ], bf16, kind="ExternalInput")
    wu_d = nc.dram_tensor("wu", [128, 8, DF], bf16, kind="ExternalInput")
    wv_d = nc.dram_tensor("wv", [128, 8, DF], bf16, kind="ExternalInput")
    wo_d = nc.dram_tensor("wo", [128, 16, D], bf16, kind="ExternalInput")
    wqk_d = nc.dram_tensor("wqk", [128, 8, S], bf16, kind="ExternalInput")
    bu_d = nc.dram_tensor("bu", [128, 16], f32, kind="ExternalInput")
    bqk_d = nc.dram_tensor("bqk", [128, 1], f32, kind="ExternalInput")
    bv_d = nc.dram_tensor("bv", [1, DF], bf16, kind="ExternalInput") if with_vbias else None
    boe_d = nc.dram_tensor("boe", [128, D], f32, kind="ExternalInput") if with_obias else None
    # qkg cols: 0,1 = q scale/bias (incl u_qk, 1/sqrt(S)); 2,3 = k
    # scale/bias; 4,5 = k*m0 scale/bias; 6,7 = k*m1 scale/bias
    qkg_d = nc.dram_tensor("qkg", [128, 8], f32, kind="ExternalInput")
    pairm_d = nc.dram_tensor("pairm", [128, 2], f32, kind="ExternalInput")
    mask_d = nc.dram_tensor("mask", [128, NJ], f32, kind="ExternalInput")
    ones_r_d = nc.dram_tensor("ones_r", [1, 128], bf16, kind="ExternalInput") if with_vbias else None
    ones_c_d = nc.dram_tensor("ones_c", [128, 1], bf16, kind="ExternalInput")
    y_d = nc.dram_tensor("y", [TQ, D], f32, kind="ExternalOutput")

    with tile.TileContext(nc) as tc:
        with (
            tc.tile_pool(name="res", bufs=1) as res,
            tc.tile_pool(name="bigw", bufs=1) as bigw,
            tc.tile_pool(name="ps", bufs=5, space="PSUM") as psp,
            tc.tile_pool(name="pssc", bufs=2, space="PSUM") as pssc,
            tc.tile_pool(name="psden", bufs=1, space="PSUM") as psden,
            tc.tile_pool(name="dram", bufs=1, space="DRAM") as dram,
        ):
            # ---- resident tiles ----
            # key tile a = [:, a//2, a%2, :]; key p of tile a at partition p
            v_sb = res.tile([128, NJV, 2, DF], bf16)
            uT_sb = res.tile([128, 16, TQ], bf16)     # [f%128, f//128, tq]
            qT_sb = res.tile([128, TQ], bf16)         # [s, tq]
            kT_sb = res.tile([128, NJV, 2, 128], bf16)
            bu_sb = res.tile([128, 16], f32)
            bqk_sb = res.tile([128, 1], f32)
            bv_sb = res.tile([1, DF], bf16, name="bv_sb") if with_vbias else None
            boe_sb = res.tile([128, D], f32, name="boe_sb") if with_obias else None
            qkg_sb = res.tile([128, 8], f32)
            pairm_sb = res.tile([128, 2], f32)
            mask_sb = res.tile([128, NJ], f32)
            ones_r = res.tile([1, 128], bf16, name="ones_r") if with_vbias else None
            ones_c = res.tile([128, 1], bf16)
            inv_sb = res.tile([128, 8], f32)          # 1/den per tq 128-slice

            # wv, then wo in the same slot (Tile waits for the v matmuls)
            wv_sb = bigw.tile([128, 8, DF], bf16, tag="bigw")

            # DRAM bounce buffers for the sum exchange.  in = [shard0;shard1]
            # holding own data * m_s (m = one-hot of the partner's rank), in
            # own-block-major layout: partitions [0:64) = even block 2jj,
            # [64:128) = odd block 2jj+1, chunk jj on the first free axis.
            # Rank r's RS output = partner's data (SPMD-symmetric).
            # THREE collectives (each pays a 15us fixed launch cost and
            # they serialize on the collective cores, so the composition
            # balances "first chunk lands early" vs "last chunk lands
            # before its attnv consumer"):
            #   RS0 = v f[0:1024] + k (cols 1024:1152): launches ~37us,
            #         done ~104us, just ahead of the ~112us attention start
            #   RS1 = v f[1024:1536] (done ~142), RS2 = v f[1536:2048]
            CC_W = [1152, 512, 512]
            # fc-group -> (buffer index, column offset)
            CC_MAP = {0: (0, 0), 1: (0, 512), 2: (1, 0), 3: (2, 0)}
            ccv_in = [dram.tile([2, 128, NJV, w], bf16, name=f"ccv{i}_in")
                      for i, w in enumerate(CC_W)]
            ccv_out = [dram.tile([128, NJV, w], bf16, name=f"ccv{i}_out")
                       for i, w in enumerate(CC_W)]

            with (
                tc.tile_pool(name="proj", bufs=1) as proj,
                tc.tile_pool(name="stg", bufs=4) as stg,
                tc.tile_pool(name="stgb", bufs=8) as stgb,
            ):
                xt_sb = proj.tile([128, 8, TQ], bf16)
                wqk_sb = proj.tile([128, 8, S], bf16)
                z_sb = proj.tile([128, 8, 128], f32)  # [s, tq//128, tq%128]
                wu_sb = proj.tile([128, 8, DF], bf16)
                # DMA issue order = need order; xt/wv/wu split per d-plane so
                # the first matmuls start after ~1 plane instead of MBs
                # DMA issue order = need order; xt/wv/wu split per d-plane so
                # the first matmuls start after ~1 plane instead of MBs
                nc.gpsimd.dma_start(wqk_sb[:], wqk_d[:])
                for kd in range(8):
                    nc.sync.dma_start(xt_sb[:, kd, :], xt_d[:, kd, :])
                nc.sync.dma_start(bqk_sb[:], bqk_d[:])
                nc.sync.dma_start(qkg_sb[:], qkg_d[:])
                nc.sync.dma_start(pairm_sb[:], pairm_d[:])
                if with_vbias:
                    nc.sync.dma_start(bv_sb[:], bv_d[:])
                    nc.sync.dma_start(ones_r[:], ones_r_d[:])
                nc.sync.dma_start(bu_sb[:], bu_d[:])
                for kd in range(8):
                    # split across the (idle) gpsimd and scalar queues so all
                    # planes land by ~10us and the first v chunk isn't gated
                    # on a 13us serial weight load
                    eng = nc.gpsimd if kd < 4 else nc.scalar
                    eng.dma_start(wv_sb[:, kd, :], wv_d[:, kd, :])
                nc.sync.dma_start(mask_sb[:], mask_d[:])
                nc.sync.dma_start(ones_c[:], ones_c_d[:])
                if with_obias:
                    nc.sync.dma_start(boe_sb[:], boe_d[:])
                # reduction-chain kd orders follow operand arrival: xt lands
                # serially on sync; wv alternates gpsimd(0-3)/scalar(4-7)
                Z_KD_ORDER = list(range(8))
                V_KD_ORDER = [4, 0, 5, 1, 6, 2, 7, 3]

                # ---- z = silu(Wqk^T xt^T + bqk) (own tokens); qT, k own.
                # kd-major over both psum chains: each xt plane arrival
                # feeds two matmuls, halving the DMA-paced idle ----
                zps = [psp.tile([128, 512], f32, tag="ps", name=f"zps{i}")
                       for i in range(2)]
                for ki, kd in enumerate(Z_KD_ORDER):
                    for tc_i in range(2):
                        nc.tensor.matmul(
                            zps[tc_i][:],
                            wqk_sb[:, kd, :],
                            xt_sb[:, kd, ts(tc_i, 512)],
                            start=(ki == 0),
                            stop=(ki == 7),
                        )
                for tc_i in range(2):
                    nc.scalar.activation(
                        z_sb[:, 4 * tc_i : 4 * tc_i + 4, :], zps[tc_i][:],
                        AF.Silu, bias=bqk_sb[:, 0:1],
                    )
                nc.vector.tensor_scalar(
                    qT_sb[:], z_sb[:, :, :], qkg_sb[:, 0:1], qkg_sb[:, 1:2],
                    OP.mult, OP.add,
                )
                # own k scattered into its key-tile slots: even block 2i ->
                # tile (i,0) cols [0:64); odd block 2i+1 -> tile (i,1) cols
                # [64:128)
                nc.vector.tensor_scalar(
                    kT_sb[:, :, 0, 0:64], z_sb[:, 0:NJV, 0:64],
                    qkg_sb[:, 2:3], qkg_sb[:, 3:4], OP.mult, OP.add,
                )
                nc.vector.tensor_scalar(
                    kT_sb[:, 0:NO, 1, 64:128], z_sb[:, 0:NO, 64:128],
                    qkg_sb[:, 2:3], qkg_sb[:, 3:4], OP.mult, OP.add,
                )
                # k staging (shard s gets k*m_s; scale/bias pre-masked
                # host-side in qkg cols 4..8) into chunk 0's extra columns
                for s_i in range(2):
                    kstg = stg.tile([128, NJV, 128], bf16, tag="stg",
                                    name=f"kstg{s_i}")
                    nc.vector.tensor_scalar(
                        kstg[:, :, :], z_sb[:, 0:NJV, :],
                        qkg_sb[:, 4 + 2 * s_i : 5 + 2 * s_i],
                        qkg_sb[:, 5 + 2 * s_i : 6 + 2 * s_i],
                        OP.mult, OP.add,
                    )
                    nc.gpsimd.dma_start(
                        ccv_in[0][s_i, :, :, 1024:1152], kstg[:, :, :]
                    )

                # ---- v = silu(x Wv + bv), own valid tokens -> v_sb slots.
                # Exchange chunks (f 0:1024, 1024:1536, 1536:2048) each
                # launch as soon as their slice is staged, so the last RS
                # lands before the attnv ftg-group that consumes it ----
                def v_chunk(jj, fcol):
                    ps = psp.tile([128, 512], f32, tag="ps", name="vps")
                    for ki, kd in enumerate(V_KD_ORDER):
                        nc.tensor.matmul(
                            ps[:],
                            xt_sb[:, kd, ts(jj, 128)],
                            wv_sb[:, kd, ds(fcol, 512)],
                            start=(ki == 0),
                            stop=(ki == 7 and not with_vbias),
                        )
                    if with_vbias:
                        nc.tensor.matmul(
                            ps[:],
                            ones_r[0:1, :],
                            bv_sb[0:1, ds(fcol, 512)],
                            start=False,
                            stop=True,
                        )
                    # psum partitions [0:64) = own block 2jj, [64:128) =
                    # block 2jj+1: activate each into its key-tile slot
                    nc.scalar.activation(
                        v_sb[0:64, jj, 0, ds(fcol, 512)], ps[0:64, :], AF.Silu
                    )
                    if 2 * jj + 1 < NJ:
                        nc.scalar.activation(
                            v_sb[64:128, jj, 1, ds(fcol, 512)],
                            ps[64:128, :], AF.Silu,
                        )

                def v_rs(cin, cout):
                    nc.gpsimd.collective_compute(
                        "ReduceScatter", OP.add, replica_groups=PAIRS,
                        ins=[cin[:]], outs=[cout[:]],
                    )

                def v_stage(jj, fcol, cin, col0):
                    odd = 2 * jj + 1 < NJ
                    for s_i in range(2):
                        vstg = stgb.tile(
                            [128, 512], bf16, tag="stg", name=f"vstg{s_i}"
                        )
                        nc.vector.tensor_scalar(
                            vstg[0:64, :], v_sb[0:64, jj, 0, ds(fcol, 512)],
                            pairm_sb[0:64, s_i : s_i + 1], None, OP.mult,
                        )
                        if odd:
                            nc.vector.tensor_scalar(
                                vstg[64:128, :],
                                v_sb[64:128, jj, 1, ds(fcol, 512)],
                                pairm_sb[64:128, s_i : s_i + 1], None, OP.mult,
                            )
                        else:
                            # keep the RS input finite (block NJ is not a
                            # valid key; its slot is never consumed)
                            nc.vector.memset(vstg[64:128, :], 0.0)
                        nc.sync.dma_start(
                            cin[s_i, :, jj, ds(col0, 512)], vstg[:, :]
                        )

                # fc-major 512-col compute groups; staging targets the
                # three exchange buffers, each RS emitted once its last
                # group is staged
                for fc in range(4):
                    cc_i, col0 = CC_MAP[fc]
                    for jj in range(NJV):
                        v_chunk(jj, fc * 512)
                        v_stage(jj, fc * 512, ccv_in[cc_i], col0)
                    if fc != 0:
                        v_rs(ccv_in[cc_i], ccv_out[cc_i])
                    if fc == 1:
                        # wu on the sync queue: after the f-half-0 staging
                        # (which gates RS0) but before u needs it.  The
                        # k/v readbacks are emitted at the top of the attn
                        # section so the sync queue never blocks on an RS
                        # while later staging DMAs still need it.
                        for kd in range(8):
                            nc.sync.dma_start(wu_sb[:, kd, :], wu_d[:, kd, :])

                # k + f[0:512] readbacks: emitted HERE (after every staging
                # DMA on the sync queue) so the scheduler cannot let their
                # RS0 wait block the fc2/fc3 staging that gates RS1/RS2.
                # RS0 completes ~68us; attention starts ~112us.
                nc.sync.dma_start(
                    kT_sb[:, :, 0, 64:128], ccv_out[0][:, :, 1024:1088]
                )
                nc.sync.dma_start(
                    kT_sb[:, 0:NO, 1, 0:64], ccv_out[0][:, 0:NO, 1088:1152]
                )
                nc.sync.dma_start(
                    v_sb[64:128, 0:2, 0, 0:512], ccv_out[0][0:64, 0:2, 0:512]
                )
                nc.sync.dma_start(
                    v_sb[0:64, 0:2, 1, 0:512], ccv_out[0][64:128, 0:2, 0:512]
                )
                nc.sync.dma_start(
                    v_sb[64:128, 2:NJV, 0, 0:512], ccv_out[0][0:64, 2:NJV, 0:512]
                )
                nc.sync.dma_start(
                    v_sb[0:64, 2:NO, 1, 0:512], ccv_out[0][64:128, 2:NO, 0:512]
                )

                # ---- uT = silu(Wu^T xt^T + bu)  [f, tq] ----
                for ft in range(16):
                    for qc in range(2):
                        ps = psp.tile([128, 512], f32, tag="ps")
                        for kd in range(8):
                            nc.tensor.matmul(
                                ps[:],
                                wu_sb[:, kd, ts(ft, 128)],
                                xt_sb[:, kd, ts(qc, 512)],
                                start=(kd == 0),
                                stop=(kd == 7),
                            )
                        nc.scalar.activation(
                            uT_sb[:, ft, ts(qc, 512)],
                            ps[:],
                            AF.Silu,
                            bias=bu_sb[:, ft : ft + 1],
                        )

            with (
                tc.tile_pool(name="attn", bufs=2) as attn,
                tc.tile_pool(name="yout", bufs=2) as yout,
            ):
                wo_sb = bigw.tile([128, 16, D], bf16, tag="bigw")
                pT = [
                    attn.tile([128, NJ, 512], bf16, tag="pt", name=f"pT{q}")
                    for q in range(2)
                ]
                oT = [
                    attn.tile([128, 16, 512], bf16, tag="ot", name=f"oT{q}")
                    for q in range(2)
                ]

                def kt_j(j):
                    return kT_sb[:, j // 2, j % 2, :]

                def v_j(j, ft):
                    return v_sb[:, j // 2, j % 2, ts(ft, 128)]

                def fused_scores_attnv(qc):
                    """scores(qc) interleaved with attnv(qc, ftg=0): the
                    attnv j-groups (4 matmuls each, trailing the scores by
                    3 js) fill the PE while the exps pace the pssc WAR, so
                    the Exp act-table-load latency and the per-j exp cost
                    hide under real work instead of filler matmuls.  The
                    first 4 score psums borrow the (idle here) ps pool so
                    the PE can run ahead of exp0."""
                    pss = None
                    for j in range(NJ):
                        if j < 4:
                            spool, stag = psp, "ps"
                        elif j == 4:
                            spool, stag = psden, "den"
                        else:
                            spool, stag = pssc, "pssc"
                        sc = spool.tile([128, 512], f32, tag=stag, name="scps")
                        nc.tensor.matmul(
                            sc[:], kt_j(j), qT_sb[:, ts(qc, 512)],
                            start=True, stop=True,
                        )
                        nc.scalar.activation(
                            pT[qc][:, j, :], sc[:], AF.Exp,
                            bias=mask_sb[:, j : j + 1],
                        )
                        if j == 3:
                            pss = [
                                psp.tile([128, 512], f32, tag="ps",
                                         name=f"ot_ps{i}")
                                for i in range(4)
                            ]
                        if j >= 3:
                            ja = j - 3
                            for i in range(4):
                                nc.tensor.matmul(
                                    pss[i][:], v_j(ja, i), pT[qc][:, ja, :],
                                    start=(ja == 0), stop=(ja == NJ - 1),
                                )
                    for ja in range(NJ - 3, NJ):
                        for i in range(4):
                            nc.tensor.matmul(
                                pss[i][:], v_j(ja, i), pT[qc][:, ja, :],
                                start=False, stop=(ja == NJ - 1),
                            )
                    for i in range(4):
                        nc.vector.tensor_mul(
                            oT[qc][:, i, :], pss[i][:], uT_sb[:, i, ts(qc, 512)]
                        )

                def attnv(qc, ftg):
                    # oT = (v^T pT) * uT; 4 parallel psum banks so each j's
                    # 4 matmuls start as soon as exp_j lands
                    pss = [
                        psp.tile([128, 512], f32, tag="ps", name=f"ot_ps{i}")
                        for i in range(4)
                    ]
                    for j in range(NJ):
                        for i in range(4):
                            ft = ftg * 4 + i
                            nc.tensor.matmul(
                                pss[i][:],
                                v_j(j, ft),
                                pT[qc][:, j, :],
                                start=(j == 0),
                                stop=(j == NJ - 1),
                            )
                    for i in range(4):
                        ft = ftg * 4 + i
                        nc.vector.tensor_mul(
                            oT[qc][:, ft, :], pss[i][:], uT_sb[:, ft, ts(qc, 512)]
                        )

                def dens(qc):
                    for sl in range(4):
                        dps = psden.tile([128, 1], f32, tag="den")
                        for j in range(NJ):
                            nc.tensor.matmul(
                                dps[:],
                                pT[qc][:, j, ts(sl, 128)],
                                ones_c[:, 0:1],
                                start=(j == 0),
                                stop=(j == NJ - 1),
                            )
                        nc.vector.reciprocal(
                            inv_sb[:, qc * 4 + sl : qc * 4 + sl + 1], dps[:]
                        )

                def yout_qc(qc):
                    for sl in range(4):
                        y_sb = yout.tile([128, D], f32, tag="y")
                        for dc in range(4):
                            ps = psp.tile([128, 256], f32, tag="ps", name="yps")
                            for ft in range(16):
                                nc.tensor.matmul(
                                    ps[:],
                                    oT[qc][:, ft, ts(sl, 128)],
                                    wo_sb[:, ft, ts(dc, 256)],
                                    start=(ft == 0),
                                    stop=(ft == 15),
                                )
                            nc.vector.tensor_scalar(
                                y_sb[:, ts(dc, 256)],
                                ps[:],
                                inv_sb[:, qc * 4 + sl : qc * 4 + sl + 1],
                                None,
                                OP.mult,
                            )
                            if with_obias:
                                nc.vector.tensor_add(
                                    y_sb[:, ts(dc, 256)], y_sb[:, ts(dc, 256)],
                                    boe_sb[:, ts(dc, 256)],
                                )
                            # per-256-col output DMA: earlier chunks stream
                            # out while later psum chains are still running,
                            # and the post-last-matmul tail shrinks
                            nc.sync.dma_start(
                                y_d[ds(qc * 512 + sl * 128, 128), ts(dc, 256)],
                                y_sb[:, ts(dc, 256)],
                            )

                def v_readback(fc):
                    # partner even blocks -> partitions [64:128) of even
                    # tiles, partner odd blocks -> [0:64) of odd tiles.
                    # Key tiles j0..j3 land in the first two (small) DMAs so
                    # the consuming attnv j-loop can start before the rest
                    # of the readback completes.
                    fcol = fc * 512
                    cc_i, col0 = CC_MAP[fc]
                    nc.sync.dma_start(
                        v_sb[64:128, 0:2, 0, ds(fcol, 512)],
                        ccv_out[cc_i][0:64, 0:2, ds(col0, 512)],
                    )
                    nc.sync.dma_start(
                        v_sb[0:64, 0:2, 1, ds(fcol, 512)],
                        ccv_out[cc_i][64:128, 0:2, ds(col0, 512)],
                    )
                    nc.sync.dma_start(
                        v_sb[64:128, 2:NJV, 0, ds(fcol, 512)],
                        ccv_out[cc_i][0:64, 2:NJV, ds(col0, 512)],
                    )
                    nc.sync.dma_start(
                        v_sb[0:64, 2:NO, 1, ds(fcol, 512)],
                        ccv_out[cc_i][64:128, 2:NO, ds(col0, 512)],
                    )

                fused_scores_attnv(0)
                # wo reuses the wv slot.  The tiny copies below (sourced from
                # pT0, which only exists after the Silu->Exp act-table switch)
                # pin each wo DMA chunk behind the table-switch barrier via
                # WAW -- otherwise those DMAs land before the barrier and
                # their queue slots gate the exps.
                for fg in range(4):
                    nc.scalar.activation(
                        wo_sb[0:1, 4 * fg, 0:1], pT[0][0:1, 0, 0:1], AF.Copy
                    )
                for fg in range(4):
                    nc.sync.dma_start(
                        wo_sb[:, 4 * fg : 4 * fg + 4, :],
                        wo_d[:, 4 * fg : 4 * fg + 4, :],
                    )
                v_readback(1)
                dens(0)
                fused_scores_attnv(1)
                attnv(0, 1)
                dens(1)
                attnv(1, 1)
                v_readback(2)
                attnv(0, 2)
                attnv(1, 2)
                v_readback(3)
                attnv(0, 3)
                attnv(1, 3)
                yout_qc(0)
                yout_qc(1)

    nc.compile()
    return nc


def _get_nc(NJ, with_vbias=True, with_obias=True):
    key = (NJ, with_vbias, with_obias)
    if key not in _NC:
        _NC[key] = _build_nc(*key)
    return _NC[key]


def _own_token_idx(h):
    """Own token order for half h: 64-token blocks {2a+h}, a ascending."""
    blocks = 2 * np.arange(16) + h
    return (blocks[:, None] * 64 + np.arange(64)[None, :]).ravel()


def _prep_in_maps(inputs, NJ, with_vbias=True, with_obias=True):
    x = np.ascontiguousarray(inputs["x"], dtype=np.float32)
    length = np.asarray(inputs["length"]).astype(np.int64)
    Wu = np.asarray(inputs["Wu_w"], np.float32)
    bu = np.asarray(inputs["Wu_b"], np.float32)
    Wv = np.asarray(inputs["Wv_w"], np.float32)
    bv = np.asarray(inputs["Wv_b"], np.float32)
    Wqk = np.asarray(inputs["Wqk_w"], np.float32)
    bqk = np.asarray(inputs["Wqk_b"], np.float32)
    Wo = np.asarray(inputs["Wo_w"], np.float32)
    bo = np.asarray(inputs["Wo_b"], np.float32)
    gamma = np.asarray(inputs["gamma"], np.float32)
    beta = np.asarray(inputs["beta"], np.float32)
    u_qk = np.asarray(inputs["u_qk"], np.float32)

    inv_s = np.float32(1.0 / np.sqrt(S))

    def pack_w(w, ko):  # [K, N] -> [128, ko, N] (k = o*128 + p)
        return np.ascontiguousarray(
            w.reshape(ko, 128, w.shape[1]).transpose(1, 0, 2).astype(BF16)
        )

    wu_p = pack_w(Wu, 8)
    wv_p = pack_w(Wv, 8)
    wo_p = pack_w(Wo, 16)
    wqk_p = pack_w(Wqk, 8)
    bu_p = np.ascontiguousarray(bu.reshape(16, 128).T.astype(np.float32))
    bqk_p = np.ascontiguousarray(bqk[:, None].astype(np.float32))
    bv_p = np.ascontiguousarray(bv[None, :].astype(BF16))
    boe_p = np.ascontiguousarray(np.broadcast_to(bo[None, :], (128, D)).astype(np.float32))
    ones_r = np.ones((1, 128), BF16)
    ones_c = np.ones((128, 1), BF16)

    in_maps = []
    for c in range(N_CORES):
        b, h = c // 2, c % 2
        idx = _own_token_idx(h)
        xb = x[b, idx]  # [TQ, D] own tokens, interleaved-block order
        xT = xb.T.astype(BF16)  # [D, TQ]
        xT_p = np.ascontiguousarray(xT.reshape(8, 128, TQ).transpose(1, 0, 2))
        # mask follows the per-core key-tile layout: tile a rows [0:64) =
        # global block (2a+h if a even else 2a+1-h), rows [64:128) = the
        # other one of {2a, 2a+1}
        L = int(length[b])
        mask = np.empty((128, NJ), np.float32)
        for a in range(NJ):
            g_own, g_par = 2 * a + h, 2 * a + 1 - h
            g_lo, g_hi = (g_own, g_par) if a % 2 == 0 else (g_par, g_own)
            for r0, g in ((0, g_lo), (64, g_hi)):
                tok = 64 * g + np.arange(64)
                mask[r0 : r0 + 64, a] = np.where(tok < L, 0.0, -1e30)
        # shard masks: my data goes only into the partner's RS shard
        m0, m1 = (0.0, 1.0) if h == 0 else (1.0, 0.0)
        qkg = np.stack(
            [gamma[0] * inv_s, beta[0] * inv_s + u_qk,
             gamma[1], beta[1],
             gamma[1] * m0, beta[1] * m0,
             gamma[1] * m1, beta[1] * m1], axis=1,
        ).astype(np.float32)  # [128, 8]
        pairm = np.ascontiguousarray(np.broadcast_to(
            np.array([m0, m1], np.float32), (128, 2),
        ))
        m = {
            "xt": xT_p,
            "wu": wu_p,
            "wv": wv_p,
            "wo": wo_p,
            "wqk": wqk_p,
            "bu": bu_p,
            "bqk": bqk_p,
            "qkg": qkg,
            "pairm": pairm,
            "mask": np.ascontiguousarray(mask),
            "ones_c": ones_c,
        }
        if with_vbias:
            m["bv"] = bv_p
            m["ones_r"] = ones_r
        if with_obias:
            m["boe"] = boe_p
        in_maps.append(m)
    return in_maps


def _gather(results):
    y = np.empty((B, T, D), np.float32)
    for c in range(N_CORES):
        b, h = c // 2, c % 2
        y[b, _own_token_idx(h), :] = results[c]["y"]
    return y


def _variant(inputs):
    with_vbias = bool(np.any(np.asarray(inputs["Wv_b"])))
    with_obias = bool(np.any(np.asarray(inputs["Wo_b"])))
    return with_vbias, with_obias


def _nj(inputs):
    length = np.asarray(inputs["length"]).astype(np.int64)
    return int(min(16, max(8, -(-int(length.max()) // 128))))


def _run(inputs, trace=False):
    from concourse.bass_utils import run_bass_kernel_spmd

    wv, wo = _variant(inputs)
    NJ = _nj(inputs)
    nc = _get_nc(NJ, wv, wo)
    in_maps = _prep_in_maps(inputs, NJ, wv, wo)
    res = run_bass_kernel_spmd(
        nc, in_maps, core_ids=list(range(N_CORES)), trace=trace
    )
    return _gather(res.results), res


def kernel(**inputs) -> np.ndarray:
    out, _ = _run(inputs)
    return out
